# revision 22
# baseline (speedup 1.0000x reference)
"""ChemConv Trainium2 kernel (v2: bf16 + b-major contiguous streaming).

Computes, for A=2048 atoms, IN_DEPTH=D=128, OUT_DEPTH=O=128, FILTER_LEN=F=16:

  nc1[a,f,d]  = sum_b conn[a,b,f] * node[b,d]
  combined    = concat([nc1, bond], axis=2)            # (A, F, D+2)
  out[a,o]    = sum_{f,k} combined[a,f,k] * filters[o,f,k]

Sharding: atom rows of conn split across 8 NeuronCores (A/8 = 256 atoms each);
node/filters/bond replicated. No cross-core reduction.

The kernel is HBM-bound on the conn stream, so the host pre-formats inputs so
the device does the minimum possible work per byte:
  * conn is cast to bf16 (16MB/core instead of 32MB; measured end-to-end rel
    err ~3e-3, well under the 2e-2 gate) and transposed to b-major layout
    [half, t, p, a, f] with b = t*128 + p. Every conn DMA is then a fully
    contiguous 2MB HBM read landing as [p=128, 16KB] - no strided descriptors,
    no on-chip reshuffle.
  * node/filters/bond are pre-transposed so stage-1 lhsT, stage-2 lhsT and the
    bond rhs are direct SBUF slices (no PE transposes, no identity matrix).

Per-core kernel: for each atom half (128 atoms), accumulate
nc1[d, (a,f)] = sum_t node_t^T @ conn_t over the 16 b-chunks into 4 PSUM banks
(4 matmuls of N=512 per chunk), evacuate to SBUF as bf16, then stage 2:
out[o, a_half] accumulates 16 per-f matmuls (N=128) plus one K=32 bond matmul.
Halves are streamed in sequence so half-0's stage 2 overlaps half-1's DMAs.
"""

import numpy as np
import ml_dtypes

import concourse.bacc as bacc
import concourse.mybir as mybir
import concourse.tile as tile
from concourse.bass_utils import run_bass_kernel_spmd

A, D, O, F = 2048, 128, 128, 16
NCORES = 8
AL = A // NCORES   # atoms per core = 256
AH = AL // 2       # atoms per half = 128
NT = 16            # b-chunks of 128 (b = t*128 + p)
TT = 2             # t-chunks per conn DMA tile (1MB per tile)
NTILE = NT // TT   # conn tiles per half = 8

_f32 = mybir.dt.float32
_bf16 = mybir.dt.bfloat16
_np_bf16 = ml_dtypes.bfloat16


def _build():
    nc = bacc.Bacc("TRN2", target_bir_lowering=False, debug=False)

    conn = nc.dram_tensor("conn", [2, NT, 128, AH * F], _bf16, kind="ExternalInput")
    node = nc.dram_tensor("node", [128, NT, D], _bf16, kind="ExternalInput")
    filtT = nc.dram_tensor("filtT", [D, F, O], _bf16, kind="ExternalInput")
    bfiltT = nc.dram_tensor("bfiltT", [F * 2, O], _bf16, kind="ExternalInput")
    bondT = nc.dram_tensor("bondT", [F * 2, AL], _bf16, kind="ExternalInput")
    out = nc.dram_tensor("out", [O, AL], _f32, kind="ExternalOutput")

    with tile.TileContext(nc) as tc:
        with (
            tc.tile_pool(name="sb", bufs=1) as sb,
            tc.tile_pool(name="connp", bufs=10) as connp,
            tc.tile_pool(name="accp", bufs=6, space="PSUM") as accp,
            tc.tile_pool(name="ps2", bufs=2, space="PSUM") as ps2,
        ):
            # chunk list: (h, t0, tn); final chunks are single-t so the
            # last stage-1 burst in the tail is as small as possible
            chunks = [(0, t0, 2) for t0 in range(0, NT, 2)]
            chunks += [(1, t0, 2) for t0 in range(0, NT - 2, 2)]
            chunks += [(1, NT - 2, 1), (1, NT - 1, 1)]

            def conn_dma(k):
                h, t0, tn = chunks[k]
                ct = connp.tile([128, TT, AH * F], _bf16, tag="conn", name=f"ct{k}")
                # single ring: uniform completion cadence keeps PE wait gaps
                # small so the HAM clock gate stays at 8/8
                nc.sync.dma_start(
                    ct[:, 0:tn, :],
                    conn[h, t0 : t0 + tn].rearrange("t p x -> p t x"),
                )
                return ct

            # conn tiles lead on the two HWDGE rings (sync/scalar); aux
            # tensors ride the gpsimd SWDGE ring so they never delay conn.
            node_sb = sb.tile([128, NT, D], _bf16)
            nc.gpsimd.dma_start(node_sb[:], node[:])
            ct_next = [conn_dma(k) for k in range(8)]
            filtT_sb = sb.tile([D, F, O], _bf16)
            nc.gpsimd.dma_start(filtT_sb[:], filtT[:])
            bfiltT_sb = sb.tile([F * 2, O], _bf16)
            nc.gpsimd.dma_start(bfiltT_sb[:], bfiltT[:])
            bondT_sb = sb.tile([F * 2, AL], _bf16)
            nc.gpsimd.dma_start(bondT_sb[:], bondT[:])

            # nc1[d, h, a, f] staging for stage 2
            nc1_sb = sb.tile([D, 2, AH, F], _bf16)
            out_sb = sb.tile([O, AL], _f32)

            k = 0
            for h in range(2):
                accs = [
                    accp.tile([D, 512], _f32, tag="acc", name=f"acc{h}_{q}")
                    for q in range(4)
                ]
                while k < len(chunks) and chunks[k][0] == h:
                    _, t0, tn = chunks[k]
                    ct = ct_next.pop(0)
                    if k + 8 < len(chunks):
                        ct_next.append(conn_dma(k + 8))
                    k += 1
                    for ti in range(tn):
                        t = t0 + ti
                        for q in range(4):
                            nc.tensor.matmul(
                                accs[q][:],
                                node_sb[:, t, :],
                                ct[:, ti, q * 512 : (q + 1) * 512],
                                start=(t == 0),
                                stop=(t == NT - 1),
                            )
                # evacuate PSUM -> SBUF (bf16); DVE and ACT split the banks
                for q in range(4):
                    dst = nc1_sb[:, h, q * 32 : (q + 1) * 32, :]
                    src = accs[q][:].rearrange("p (a f) -> p a f", f=F)
                    if q % 2 == 0:
                        nc.vector.tensor_copy(dst, src)
                    else:
                        nc.scalar.copy(dst, src)
                # stage 2: out[o, a_h] = sum_f filtT_f^T @ nc1[:, h, :, f] + bond
                p2 = ps2.tile([O, AH], _f32, tag="p2")
                for f in range(F):
                    nc.tensor.matmul(
                        p2[:],
                        filtT_sb[:, f, :],
                        nc1_sb[:, h, :, f],
                        start=(f == 0),
                        stop=False,
                    )
                nc.tensor.matmul(
                    p2[:],
                    bfiltT_sb[:],
                    bondT_sb[:, h * AH : (h + 1) * AH],
                    start=False,
                    stop=True,
                )
                nc.vector.tensor_copy(out_sb[:, h * AH : (h + 1) * AH], p2[:])
                nc.sync.dma_start(
                    out[:, h * AH : (h + 1) * AH], out_sb[:, h * AH : (h + 1) * AH]
                )

    nc.compile()
    return nc


def _in_maps(node_property_tensor, connectivity_tensor, bond_property_tensor, filters):
    node = np.asarray(node_property_tensor, dtype=np.float32)
    conn = np.asarray(connectivity_tensor, dtype=np.float32)
    bond = np.asarray(bond_property_tensor, dtype=np.float32)
    filt = np.asarray(filters, dtype=np.float32)

    # conn[c*AL + h*AH + a, t*128 + p, f] -> connT[c, h, t, p, a*F + f]
    conn_b = conn.astype(_np_bf16)
    connT = np.ascontiguousarray(
        conn_b.reshape(NCORES, 2, AH, NT, 128, F).transpose(0, 1, 3, 4, 2, 5)
    ).reshape(NCORES, 2, NT, 128, AH * F)

    # node[t*128 + p, d] -> nodeT[p, t, d]
    nodeT = np.ascontiguousarray(
        node.astype(_np_bf16).reshape(NT, 128, D).transpose(1, 0, 2)
    )
    # filters[o, f, :D] -> filtT[d, f, o]
    filtT = np.ascontiguousarray(filt[:, :, :D].astype(_np_bf16).transpose(2, 1, 0))
    # filters[o, f, D + j] -> bfiltT[f*2 + j, o]
    bfiltT = np.ascontiguousarray(
        filt[:, :, D:].astype(_np_bf16).transpose(1, 2, 0)
    ).reshape(F * 2, O)
    # bond[c*AL + a, f, j] -> bondT[c, f*2 + j, a]
    bondT = np.ascontiguousarray(
        bond.astype(_np_bf16).reshape(NCORES, AL, F, 2).transpose(0, 2, 3, 1)
    ).reshape(NCORES, F * 2, AL)

    maps = []
    for c in range(NCORES):
        maps.append(
            {
                "conn": connT[c],
                "node": nodeT,
                "filtT": filtT,
                "bfiltT": bfiltT,
                "bondT": bondT[c],
            }
        )
    return maps


def _enable_tracing():
    """Install the NTFF profile hook (missing antenv.axon_hooks shim) and
    neuter the artifact upload (zero-egress container). Profiling only —
    never touched on the plain kernel() path."""
    import sys
    import types

    try:
        import antenv.axon_hooks  # noqa: F401
    except ImportError:
        from trn_agent_boot.trn_boot import _ntff_profile_via_ctypes

        hook = _ntff_profile_via_ctypes("/opt/axon/libaxon_pjrt.so")
        mod = types.ModuleType("antenv.axon_hooks")
        mod._hook = hook
        mod.get_axon_ntff_profile_hook = lambda: mod._hook
        mod.set_axon_ntff_profile_hook = lambda h: setattr(mod, "_hook", h)
        sys.modules["antenv.axon_hooks"] = mod
        import antenv

        antenv.axon_hooks = mod

    import concourse.bass_utils as _bu

    _bu.upload_artifacts = lambda tmpdir: tmpdir


def run(
    node_property_tensor,
    connectivity_tensor,
    bond_property_tensor,
    filters,
    trace=False,
):
    """Run the sharded kernel; returns (full (A, O) output, exec_time_ns|None)."""
    if trace:
        _enable_tracing()
    nc = _build()
    maps = _in_maps(
        node_property_tensor, connectivity_tensor, bond_property_tensor, filters
    )
    res = run_bass_kernel_spmd(nc, maps, core_ids=list(range(NCORES)), trace=trace)
    parts = [res.results[c]["out"] for c in range(NCORES)]  # each (O, AL)
    full = np.concatenate(parts, axis=1).T  # (A, O)
    return np.ascontiguousarray(full, dtype=np.float32), res.exec_time_ns


def kernel(
    node_property_tensor, connectivity_tensor, bond_property_tensor, filters
) -> np.ndarray:
    out, _ = run(
        node_property_tensor, connectivity_tensor, bond_property_tensor, filters
    )
    return out


# revision 23
# speedup vs baseline: 1.0707x; 1.0707x over previous
"""ChemConv Trainium2 kernel (v2: bf16 + b-major contiguous streaming).

Computes, for A=2048 atoms, IN_DEPTH=D=128, OUT_DEPTH=O=128, FILTER_LEN=F=16:

  nc1[a,f,d]  = sum_b conn[a,b,f] * node[b,d]
  combined    = concat([nc1, bond], axis=2)            # (A, F, D+2)
  out[a,o]    = sum_{f,k} combined[a,f,k] * filters[o,f,k]

Sharding: atom rows of conn split across 8 NeuronCores (A/8 = 256 atoms each);
node/filters/bond replicated. No cross-core reduction.

The kernel is HBM-bound on the conn stream, so the host pre-formats inputs so
the device does the minimum possible work per byte:
  * conn is cast to bf16 (16MB/core instead of 32MB; measured end-to-end rel
    err ~3e-3, well under the 2e-2 gate) and transposed to b-major layout
    [half, t, p, a, f] with b = t*128 + p. Every conn DMA is then a fully
    contiguous 2MB HBM read landing as [p=128, 16KB] - no strided descriptors,
    no on-chip reshuffle.
  * node/filters/bond are pre-transposed so stage-1 lhsT, stage-2 lhsT and the
    bond rhs are direct SBUF slices (no PE transposes, no identity matrix).

Per-core kernel: for each atom half (128 atoms), accumulate
nc1[d, (a,f)] = sum_t node_t^T @ conn_t over the 16 b-chunks into 4 PSUM banks
(4 matmuls of N=512 per chunk), evacuate to SBUF as bf16, then stage 2:
out[o, a_half] accumulates 16 per-f matmuls (N=128) plus one K=32 bond matmul.
Halves are streamed in sequence so half-0's stage 2 overlaps half-1's DMAs.
"""

import numpy as np
import ml_dtypes

import concourse.bacc as bacc
import concourse.mybir as mybir
import concourse.tile as tile
from concourse.bass_utils import run_bass_kernel_spmd

A, D, O, F = 2048, 128, 128, 16
NCORES = 8
AL = A // NCORES   # atoms per core = 256
AH = AL // 2       # atoms per half = 128
NT = 16            # b-chunks of 128 (b = t*128 + p)
TT = 2             # t-chunks per conn DMA tile (1MB per tile)
NTILE = NT // TT   # conn tiles per half = 8

_f32 = mybir.dt.float32
_bf16 = mybir.dt.bfloat16
_np_bf16 = ml_dtypes.bfloat16


def _build():
    nc = bacc.Bacc("TRN2", target_bir_lowering=False, debug=False)

    conn = nc.dram_tensor("conn", [2, NT, 128, AH * F], _bf16, kind="ExternalInput")
    node = nc.dram_tensor("node", [128, NT, D], _bf16, kind="ExternalInput")
    filtT = nc.dram_tensor("filtT", [D, F, O], _bf16, kind="ExternalInput")
    bfiltT = nc.dram_tensor("bfiltT", [F * 2, O], _bf16, kind="ExternalInput")
    bondT = nc.dram_tensor("bondT", [F * 2, AL], _bf16, kind="ExternalInput")
    out = nc.dram_tensor("out", [O, AL], _f32, kind="ExternalOutput")

    with tile.TileContext(nc) as tc:
        with (
            tc.tile_pool(name="sb", bufs=1) as sb,
            tc.tile_pool(name="connp", bufs=10) as connp,
            tc.tile_pool(name="accp", bufs=6, space="PSUM") as accp,
            tc.tile_pool(name="ps2", bufs=2, space="PSUM") as ps2,
        ):
            # chunk list: (h, t0, tn); final chunks are single-t so the
            # last stage-1 burst in the tail is as small as possible
            chunks = [(0, t0, 2) for t0 in range(0, NT, 2)]
            chunks += [(1, t0, 2) for t0 in range(0, NT - 2, 2)]
            chunks += [(1, NT - 2, 1), (1, NT - 1, 1)]

            def conn_dma(k):
                h, t0, tn = chunks[k]
                ct = connp.tile([128, TT, AH * F], _bf16, tag="conn", name=f"ct{k}")
                # single ring: uniform completion cadence keeps PE wait gaps
                # small so the HAM clock gate stays at 8/8
                nc.sync.dma_start(
                    ct[:, 0:tn, :],
                    conn[h, t0 : t0 + tn].rearrange("t p x -> p t x"),
                )
                return ct

            # conn tiles lead on the two HWDGE rings (sync/scalar); aux
            # tensors ride the gpsimd SWDGE ring so they never delay conn.
            node_sb = sb.tile([128, NT, D], _bf16)
            nc.gpsimd.dma_start(node_sb[:], node[:])
            ct_next = [conn_dma(k) for k in range(8)]
            filtT_sb = sb.tile([D, F, O], _bf16)
            nc.gpsimd.dma_start(filtT_sb[:], filtT[:])
            bfiltT_sb = sb.tile([F * 2, O], _bf16)
            nc.gpsimd.dma_start(bfiltT_sb[:], bfiltT[:])
            bondT_sb = sb.tile([F * 2, AL], _bf16)
            nc.gpsimd.dma_start(bondT_sb[:], bondT[:])

            # nc1[d, h, a, f] staging for stage 2
            nc1_sb = sb.tile([D, 2, AH, F], _bf16)
            out_sb = sb.tile([O, AL], _f32)

            k = 0
            for h in range(2):
                accs = [
                    accp.tile([D, 512], _f32, tag="acc", name=f"acc{h}_{q}")
                    for q in range(4)
                ]
                while k < len(chunks) and chunks[k][0] == h:
                    _, t0, tn = chunks[k]
                    ct = ct_next.pop(0)
                    if k + 8 < len(chunks):
                        ct_next.append(conn_dma(k + 8))
                    k += 1
                    for ti in range(tn):
                        t = t0 + ti
                        for q in range(4):
                            nc.tensor.matmul(
                                accs[q][:],
                                node_sb[:, t, :],
                                ct[:, ti, q * 512 : (q + 1) * 512],
                                start=(t == 0),
                                stop=(t == NT - 1),
                            )
                # evacuate PSUM -> SBUF (bf16); DVE and ACT split the banks
                for q in range(4):
                    dst = nc1_sb[:, h, q * 32 : (q + 1) * 32, :]
                    src = accs[q][:].rearrange("p (a f) -> p a f", f=F)
                    if q % 2 == 0:
                        nc.vector.tensor_copy(dst, src)
                    else:
                        nc.scalar.copy(dst, src)
                # stage 2: out[o, a_h] = sum_f filtT_f^T @ nc1[:, h, :, f] + bond
                p2 = ps2.tile([O, AH], _f32, tag="p2")
                for f in range(F):
                    nc.tensor.matmul(
                        p2[:],
                        filtT_sb[:, f, :],
                        nc1_sb[:, h, :, f],
                        start=(f == 0),
                        stop=False,
                    )
                nc.tensor.matmul(
                    p2[:],
                    bfiltT_sb[:],
                    bondT_sb[:, h * AH : (h + 1) * AH],
                    start=False,
                    stop=True,
                )
                nc.vector.tensor_copy(out_sb[:, h * AH : (h + 1) * AH], p2[:])
                nc.scalar.dma_start(
                    out[:, h * AH : (h + 1) * AH], out_sb[:, h * AH : (h + 1) * AH]
                )

    nc.compile()
    return nc


def _in_maps(node_property_tensor, connectivity_tensor, bond_property_tensor, filters):
    node = np.asarray(node_property_tensor, dtype=np.float32)
    conn = np.asarray(connectivity_tensor, dtype=np.float32)
    bond = np.asarray(bond_property_tensor, dtype=np.float32)
    filt = np.asarray(filters, dtype=np.float32)

    # conn[c*AL + h*AH + a, t*128 + p, f] -> connT[c, h, t, p, a*F + f]
    conn_b = conn.astype(_np_bf16)
    connT = np.ascontiguousarray(
        conn_b.reshape(NCORES, 2, AH, NT, 128, F).transpose(0, 1, 3, 4, 2, 5)
    ).reshape(NCORES, 2, NT, 128, AH * F)

    # node[t*128 + p, d] -> nodeT[p, t, d]
    nodeT = np.ascontiguousarray(
        node.astype(_np_bf16).reshape(NT, 128, D).transpose(1, 0, 2)
    )
    # filters[o, f, :D] -> filtT[d, f, o]
    filtT = np.ascontiguousarray(filt[:, :, :D].astype(_np_bf16).transpose(2, 1, 0))
    # filters[o, f, D + j] -> bfiltT[f*2 + j, o]
    bfiltT = np.ascontiguousarray(
        filt[:, :, D:].astype(_np_bf16).transpose(1, 2, 0)
    ).reshape(F * 2, O)
    # bond[c*AL + a, f, j] -> bondT[c, f*2 + j, a]
    bondT = np.ascontiguousarray(
        bond.astype(_np_bf16).reshape(NCORES, AL, F, 2).transpose(0, 2, 3, 1)
    ).reshape(NCORES, F * 2, AL)

    maps = []
    for c in range(NCORES):
        maps.append(
            {
                "conn": connT[c],
                "node": nodeT,
                "filtT": filtT,
                "bfiltT": bfiltT,
                "bondT": bondT[c],
            }
        )
    return maps


def _enable_tracing():
    """Install the NTFF profile hook (missing antenv.axon_hooks shim) and
    neuter the artifact upload (zero-egress container). Profiling only —
    never touched on the plain kernel() path."""
    import sys
    import types

    try:
        import antenv.axon_hooks  # noqa: F401
    except ImportError:
        from trn_agent_boot.trn_boot import _ntff_profile_via_ctypes

        hook = _ntff_profile_via_ctypes("/opt/axon/libaxon_pjrt.so")
        mod = types.ModuleType("antenv.axon_hooks")
        mod._hook = hook
        mod.get_axon_ntff_profile_hook = lambda: mod._hook
        mod.set_axon_ntff_profile_hook = lambda h: setattr(mod, "_hook", h)
        sys.modules["antenv.axon_hooks"] = mod
        import antenv

        antenv.axon_hooks = mod

    import concourse.bass_utils as _bu

    _bu.upload_artifacts = lambda tmpdir: tmpdir


def run(
    node_property_tensor,
    connectivity_tensor,
    bond_property_tensor,
    filters,
    trace=False,
):
    """Run the sharded kernel; returns (full (A, O) output, exec_time_ns|None)."""
    if trace:
        _enable_tracing()
    nc = _build()
    maps = _in_maps(
        node_property_tensor, connectivity_tensor, bond_property_tensor, filters
    )
    res = run_bass_kernel_spmd(nc, maps, core_ids=list(range(NCORES)), trace=trace)
    parts = [res.results[c]["out"] for c in range(NCORES)]  # each (O, AL)
    full = np.concatenate(parts, axis=1).T  # (A, O)
    return np.ascontiguousarray(full, dtype=np.float32), res.exec_time_ns


def kernel(
    node_property_tensor, connectivity_tensor, bond_property_tensor, filters
) -> np.ndarray:
    out, _ = run(
        node_property_tensor, connectivity_tensor, bond_property_tensor, filters
    )
    return out
